# revision 5
# baseline (speedup 1.0000x reference)
"""Bahdanau-attention GRU cell fused Trainium2 kernel.

Sharding: data-parallel over batch across 8 NeuronCores (4 batch rows per
core, weights replicated, no collectives).

Math per core (b=4 local batch rows, T=2048, F=U=512):
  pre^T[u,t]  = Ua^T ann^T + (Wa^T h + Wa_bias + Ua_bias)      (PE, bf16)
  tanh fused on ScalarE with per-partition bias
  scores      = Va . tanh(pre)      (PE matmul, Va replicated across M=128)
  p = exp(scores) (no max-sub; |scores| <= sum|Va| ~ 20, safe in fp32),
  Z via activation accum_out
  c_unnorm^T  = sum_t ann^T[f,t] * p[t]   (DVE tensor_tensor_reduce)
  GRU gates: one PSUM accumulation of x@K + c@AK + h@RK[:,:2U] + biases,
  hard-sigmoid/tanh epilogue, h_new = z*h + (1-z)*hh
"""

import sys

if "/opt/trn_rl_repo" not in sys.path:
    sys.path.insert(0, "/opt/trn_rl_repo")

import numpy as np

import concourse.bass as bass
import concourse.tile as tile
from concourse import bacc, bass_utils, mybir
from concourse.masks import make_identity

F32 = mybir.dt.float32
BF16 = mybir.dt.bfloat16
AF = mybir.ActivationFunctionType
ALU = mybir.AluOpType

B, T, F, U = 32, 2048, 512, 512
NCORES = 8
BL = B // NCORES          # 4 local batch rows
TT = 512                  # T-tile (free dim of matmuls)
NTT = T // TT             # 4
NS = TT // 128            # 4 t-subtiles per T-tile
NFB = F // 128            # 4 f blocks
NUB = U // 128            # 4 u blocks
U3 = 3 * U


def build(profile_friendly=False):
    nc = bacc.Bacc("TRN2", target_bir_lowering=False, debug=False)

    def din(name, shape):
        return nc.dram_tensor(name, shape, F32, kind="ExternalInput").ap()

    d_x = din("x", [BL, F])
    d_h = din("h", [BL, U])
    d_ann = din("annotations", [BL, T, F])
    d_k = din("kernel", [F, U3])
    d_rk = din("recurrent_kernel", [U, U3])
    d_ak = din("attention_kernel", [F, U3])
    d_wa = din("Wa", [U, U])
    d_ua = din("Ua", [F, U])
    d_va = din("Va", [U])
    d_bias = din("bias", [U3])
    d_abias = din("attention_bias", [U3])
    d_wab = din("Wa_bias", [U])
    d_uab = din("Ua_bias", [U])
    d_out = nc.dram_tensor("h_new", [BL, U], F32, kind="ExternalOutput").ap()

    with tile.TileContext(nc) as tc:
        with (
            tc.tile_pool(name="const", bufs=1) as const,
            tc.tile_pool(name="annio", bufs=3) as annio,
            tc.tile_pool(name="annT_p", bufs=2) as annT_p,
            tc.tile_pool(name="tanh_p", bufs=2) as tanh_p,
            tc.tile_pool(name="pbc_p", bufs=2) as pbc_p,
            tc.tile_pool(name="scr_p", bufs=2) as scr_p,
            tc.tile_pool(name="ps_tp", bufs=2, space="PSUM") as ps_tp,
            tc.tile_pool(name="ps_pre", bufs=2, space="PSUM") as ps_pre,
            tc.tile_pool(name="ps_sc", bufs=2, space="PSUM") as ps_sc,
        ):
            # ---------------- constants / weights ----------------
            ident = const.tile([128, 128], BF16)
            make_identity(nc, ident[:])

            ones4 = const.tile([1, BL], BF16)
            nc.vector.memset(ones4[:], 1.0)

            ua_sb = const.tile([128, NFB, U], BF16)
            nc.gpsimd.dma_start(
                out=ua_sb[:], in_=d_ua.rearrange("(fb p) u -> p fb u", p=128)
            )
            wa_sb = const.tile([128, NUB, U], BF16)
            nc.gpsimd.dma_start(
                out=wa_sb[:], in_=d_wa.rearrange("(jb p) u -> p jb u", p=128)
            )
            va_row = const.tile([1, U], BF16)
            nc.gpsimd.dma_start(out=va_row[:], in_=d_va.rearrange("(a u) -> a u", a=1))
            wab_row = const.tile([1, U], BF16)
            nc.gpsimd.dma_start(out=wab_row[:], in_=d_wab.rearrange("(a u) -> a u", a=1))
            uab_row = const.tile([1, U], BF16)
            nc.gpsimd.dma_start(out=uab_row[:], in_=d_uab.rearrange("(a u) -> a u", a=1))
            bias_row = const.tile([1, U3], BF16)
            nc.gpsimd.dma_start(out=bias_row[:], in_=d_bias.rearrange("(a u) -> a u", a=1))
            abias_row = const.tile([1, U3], BF16)
            nc.gpsimd.dma_start(out=abias_row[:], in_=d_abias.rearrange("(a u) -> a u", a=1))

            x_bf = const.tile([BL, F], BF16)
            nc.gpsimd.dma_start(out=x_bf[:], in_=d_x)
            h_bf = const.tile([BL, U], BF16)
            nc.gpsimd.dma_start(out=h_bf[:], in_=d_h)
            h_f32 = const.tile([BL, U], F32)
            nc.sync.dma_start(out=h_f32[:], in_=d_h)

            # GRU weights (loaded in chunks, interleaved into the main loop
            # below so the big annotation streams are not delayed).
            k_sb = const.tile([128, NFB, U3], BF16)
            rk_sb = const.tile([128, NUB, U3], BF16)
            ak_sb = const.tile([128, NFB, U3], BF16)
            k_r = d_k.rearrange("(fb p) u -> p fb u", p=128)
            rk_r = d_rk.rearrange("(fb p) u -> p fb u", p=128)
            ak_r = d_ak.rearrange("(fb p) u -> p fb u", p=128)
            gru_w_chunks = []
            for fb in range(NFB):
                gru_w_chunks.append((k_sb, k_r, fb))
                gru_w_chunks.append((rk_sb, rk_r, fb))
                gru_w_chunks.append((ak_sb, ak_r, fb))

            # VaT replicated: VaT_rep[p, ub, j] = Va[ub*128+p] for all j
            va_rep = const.tile([128, NUB, 128], BF16)
            for ub in range(NUB):
                tp = ps_tp.tile([128, 128], BF16, tag="tp")
                nc.tensor.transpose(
                    tp[:, 0:1], va_row[0:1, 128 * ub : 128 * (ub + 1)], ident[0:1, 0:1]
                )
                nc.vector.tensor_copy(
                    va_rep[:, ub, :], tp[:, 0:1].to_broadcast([128, 128])
                )

            # x^T, h^T  (transpose [4,128] chunks -> [128,4])
            xT = const.tile([128, NFB, BL], BF16)
            hT = const.tile([128, NUB, BL], BF16)
            for jb in range(NFB):
                tp = ps_tp.tile([128, 128], BF16, tag="tp")
                nc.tensor.transpose(
                    tp[:, 0:BL], x_bf[0:BL, 128 * jb : 128 * (jb + 1)], ident[0:BL, 0:BL]
                )
                nc.any.tensor_copy(xT[:, jb, :], tp[:, 0:BL])
            for jb in range(NUB):
                tp = ps_tp.tile([128, 128], BF16, tag="tp")
                nc.tensor.transpose(
                    tp[:, 0:BL], h_bf[0:BL, 128 * jb : 128 * (jb + 1)], ident[0:BL, 0:BL]
                )
                nc.any.tensor_copy(hT[:, jb, :], tp[:, 0:BL])

            # q^T[u, b] = Wa^T h^T + Wa_bias + Ua_bias
            qT = const.tile([128, NUB, BL], F32)
            for ub in range(NUB):
                qp = ps_pre.tile([128, TT], F32, tag="pre")
                for jb in range(NUB):
                    nc.tensor.matmul(
                        qp[:, 0:BL],
                        wa_sb[:, jb, 128 * ub : 128 * (ub + 1)],
                        hT[:, jb, :],
                        start=(jb == 0),
                        stop=False,
                    )
                nc.tensor.matmul(
                    qp[:, 0:BL],
                    wab_row[0:1, 128 * ub : 128 * (ub + 1)],
                    ones4[:],
                    start=False,
                    stop=False,
                )
                nc.tensor.matmul(
                    qp[:, 0:BL],
                    uab_row[0:1, 128 * ub : 128 * (ub + 1)],
                    ones4[:],
                    start=False,
                    stop=True,
                )
                nc.any.tensor_copy(qT[:, ub, :], qp[:, 0:BL])

            # softmax normalizer partials (per (b,tt)) and context partials
            ztile = const.tile([128, BL * NTT], F32)
            cpart = const.tile([128, NFB, NTT, BL], F32)

            # ---------------- main attention loop ----------------
            ann_r = d_ann.rearrange("b (tt s p) f -> b tt s p f", p=128, s=NS)
            for b in range(BL):
                for tt in range(NTT):
                    it = b * NTT + tt
                    # stream in the natural-layout tile, cast fp32->bf16
                    a_nat = annio.tile([128, NS, F], BF16, tag="ann_nat")
                    nc.gpsimd.dma_start(
                        out=a_nat[:],
                        in_=ann_r[b, tt].rearrange("s p f -> p s f"),
                    )
                    # interleave one GRU weight chunk DMA per iteration
                    if it < len(gru_w_chunks):
                        wsb, wr, fb = gru_w_chunks[it]
                        nc.gpsimd.dma_start(out=wsb[:, fb, :], in_=wr[:, fb, :])

                    # transpose to [f, t]
                    a_T = annT_p.tile([128, NFB, TT], BF16, tag="annT")
                    for s in range(NS):
                        for fb in range(NFB):
                            tp = ps_tp.tile([128, 128], BF16, tag="tp")
                            nc.tensor.transpose(
                                tp[:],
                                a_nat[:, s, 128 * fb : 128 * (fb + 1)],
                                ident[:],
                            )
                            nc.any.tensor_copy(
                                a_T[:, fb, 128 * s : 128 * (s + 1)], tp[:]
                            )

                    # pre^T = Ua^T ann^T ; tanh(+q) fused on ScalarE
                    t_T = tanh_p.tile([128, NUB, TT], BF16, tag="tanhT")
                    for ub in range(NUB):
                        pp = ps_pre.tile([128, TT], F32, tag="pre")
                        for fb in range(NFB):
                            nc.tensor.matmul(
                                pp[:],
                                ua_sb[:, fb, 128 * ub : 128 * (ub + 1)],
                                a_T[:, fb, :],
                                start=(fb == 0),
                                stop=(fb == NFB - 1),
                            )
                        nc.scalar.activation(
                            t_T[:, ub, :], pp[:], AF.Tanh, bias=qT[:, ub, b : b + 1]
                        )

                    # scores (replicated across partitions) + exp + Z partial
                    sp = ps_sc.tile([128, TT], F32, tag="sc")
                    for ub in range(NUB):
                        nc.tensor.matmul(
                            sp[:],
                            va_rep[:, ub, :],
                            t_T[:, ub, :],
                            start=(ub == 0),
                            stop=(ub == NUB - 1),
                        )
                    p_bc = pbc_p.tile([128, TT], BF16, tag="pbc")
                    nc.scalar.activation(
                        p_bc[:], sp[:], AF.Exp, accum_out=ztile[:, it : it + 1]
                    )

                    # context partials: cpart[f, fb, tt, b] = sum_t annT * p
                    # (DVE multiply, then ScalarE Identity pass whose
                    # accum_out sums along the free axis)
                    for fb in range(NFB):
                        scr = scr_p.tile([128, TT], BF16, tag="scr")
                        nc.vector.tensor_mul(scr[:], a_T[:, fb, :], p_bc[:])
                        scr2 = scr_p.tile([128, TT], BF16, tag="scr2")
                        nc.scalar.activation(
                            scr2[:], scr[:], AF.Identity,
                            accum_out=cpart[:, fb, tt, b : b + 1],
                        )

            # ---------------- softmax normalization ----------------
            zsum = const.tile([128, BL], F32)
            rz = const.tile([128, BL], F32)
            for b in range(BL):
                nc.vector.reduce_sum(
                    zsum[:, b : b + 1],
                    ztile[:, b * NTT : (b + 1) * NTT],
                    axis=mybir.AxisListType.X,
                )
            nc.vector.reciprocal(rz[:], zsum[:])

            cT = const.tile([128, NFB, BL], BF16)
            csum = const.tile([128, NFB, BL], F32)
            for b in range(BL):
                for fb in range(NFB):
                    nc.vector.reduce_sum(
                        csum[:, fb, b : b + 1],
                        cpart[:, fb, :, b],
                        axis=mybir.AxisListType.X,
                    )
                nc.vector.tensor_scalar(
                    out=cT[:, :, b],
                    in0=csum[:, :, b],
                    scalar1=rz[:, b : b + 1],
                    scalar2=None,
                    op0=ALU.mult,
                )

            # ---------------- GRU ----------------
            # gate pre-activations: g[:, n] = x@K + c_att@AK + bias + abias
            # (+ h@RK for the z/r blocks)
            g_ps = []
            for nb in range(3):
                if nb % 2 == 0:
                    gp = ps_pre.tile([4, TT], F32, tag="pre", name=f"g_ps{nb}")
                else:
                    gp = ps_sc.tile([4, TT], F32, tag="sc", name=f"g_ps{nb}")
                n0 = nb * TT
                for fb in range(NFB):
                    nc.tensor.matmul(
                        gp[:],
                        xT[:, fb, :],
                        k_sb[:, fb, n0 : n0 + TT],
                        start=(fb == 0),
                        stop=False,
                    )
                for fb in range(NFB):
                    nc.tensor.matmul(
                        gp[:],
                        cT[:, fb, :],
                        ak_sb[:, fb, n0 : n0 + TT],
                        start=False,
                        stop=False,
                    )
                if nb < 2:
                    for ub in range(NUB):
                        nc.tensor.matmul(
                            gp[:],
                            hT[:, ub, :],
                            rk_sb[:, ub, n0 : n0 + TT],
                            start=False,
                            stop=False,
                        )
                nc.tensor.matmul(
                    gp[:], ones4[:], bias_row[0:1, n0 : n0 + TT], start=False, stop=False
                )
                nc.tensor.matmul(
                    gp[:], ones4[:], abias_row[0:1, n0 : n0 + TT], start=False, stop=True
                )
                g_ps.append(gp)

            # hard sigmoid gates
            def hard_sigmoid(dst, src):
                tmp = const.tile([BL, U], F32, name=f"hs_tmp_{dst.name}")
                nc.vector.tensor_scalar(
                    out=tmp[:], in0=src, scalar1=0.2, scalar2=0.5, op0=ALU.mult,
                    op1=ALU.add,
                )
                nc.vector.tensor_scalar(
                    out=dst, in0=tmp[:], scalar1=0.0, scalar2=1.0, op0=ALU.max,
                    op1=ALU.min,
                )

            z_sb = const.tile([BL, U], F32)
            r_sb = const.tile([BL, U], F32)
            hard_sigmoid(z_sb[:], g_ps[0][:])
            hard_sigmoid(r_sb[:], g_ps[1][:])

            rh_bf = const.tile([BL, U], BF16)
            nc.vector.tensor_tensor(
                out=rh_bf[:], in0=r_sb[:], in1=h_f32[:], op=ALU.mult
            )
            rhT = const.tile([128, NUB, BL], BF16)
            for ub in range(NUB):
                tp = ps_tp.tile([128, 128], BF16, tag="tp")
                nc.tensor.transpose(
                    tp[:, 0:BL], rh_bf[0:BL, 128 * ub : 128 * (ub + 1)],
                    ident[0:BL, 0:BL],
                )
                nc.any.tensor_copy(rhT[:, ub, :], tp[:, 0:BL])

            hh_ps = ps_tp.tile([4, TT], F32, tag="tp")
            for ub in range(NUB):
                nc.tensor.matmul(
                    hh_ps[:],
                    rhT[:, ub, :],
                    rk_sb[:, ub, 2 * U : 3 * U],
                    start=(ub == 0),
                    stop=(ub == NUB - 1),
                )

            xh_sb = const.tile([BL, U], F32)
            nc.any.tensor_copy(xh_sb[:], g_ps[2][:])
            hh_pre = const.tile([BL, U], F32)
            nc.vector.tensor_tensor(
                out=hh_pre[:], in0=xh_sb[:], in1=hh_ps[:], op=ALU.add
            )
            hh = const.tile([BL, U], F32)
            nc.scalar.activation(hh[:], hh_pre[:], AF.Tanh)

            # h_new = hh + z * (h - hh)
            d_sb = const.tile([BL, U], F32)
            nc.vector.tensor_tensor(out=d_sb[:], in0=h_f32[:], in1=hh[:], op=ALU.subtract)
            zd = const.tile([BL, U], F32)
            nc.vector.tensor_tensor(out=zd[:], in0=z_sb[:], in1=d_sb[:], op=ALU.mult)
            out_sb = const.tile([BL, U], F32)
            nc.vector.tensor_tensor(out=out_sb[:], in0=hh[:], in1=zd[:], op=ALU.add)
            nc.sync.dma_start(out=d_out, in_=out_sb[:])

    nc.compile()
    return nc


_NC = None


def _get_nc():
    global _NC
    if _NC is None:
        _NC = build()
    return _NC


def kernel(**inputs):
    nc = _get_nc()
    shared = {
        k: np.ascontiguousarray(np.asarray(inputs[k], np.float32))
        for k in (
            "kernel", "recurrent_kernel", "attention_kernel", "Wa", "Ua", "Va",
            "bias", "attention_bias", "Wa_bias", "Ua_bias",
        )
    }
    in_maps = []
    for c in range(NCORES):
        sl = slice(c * BL, (c + 1) * BL)
        m = dict(shared)
        m["x"] = np.ascontiguousarray(np.asarray(inputs["x"], np.float32)[sl])
        m["h"] = np.ascontiguousarray(np.asarray(inputs["h"], np.float32)[sl])
        m["annotations"] = np.ascontiguousarray(
            np.asarray(inputs["annotations"], np.float32)[sl]
        )
        in_maps.append(m)
    res = bass_utils.run_bass_kernel_spmd(nc, in_maps, core_ids=list(range(NCORES)))
    return np.concatenate([r["h_new"] for r in res.results], axis=0)
